# revision 25
# baseline (speedup 1.0000x reference)
"""AFM forward on 8 TRN2 NeuronCores: dma_gather (256B quarter-row blocks) + on-chip select.

Math (softmax over the reference's size-1 axis is identically 1, so the
attention branch is dead): out[b] = sigmoid(0.5*w*(||S_b||^2 - Q_b) + bias) with
S_b = sum_f e_{b,f}, Q_b = sum_f ||e_{b,f}||^2.

Gather: per field f one InstDMAGatherAnt with int16 quarter-row indices
qid = id>>2 into table viewed as [25000, 64] 256B rows; each lookup lands
the 4-row block containing its embedding row.  Select on-chip with a 0/1
mask M4[(f,t),r] = (id & 3 == r).

Layouts: batch b = t*128 + p (t in [0,4)).  G/GM [128, f*256 + t*64 + r*16+k].
Host transposes the [128,4] output back to [512,1].
"""

import numpy as np

import concourse.bacc as bacc
import concourse.bass as bass
import concourse.mybir as mybir
from concourse import library_config
from concourse.bass_utils import run_bass_kernel_spmd

N_CORES = 8
B = 4096
NF = 26
EMB = 16
VOCAB = 100000
P = 128
B_SHARD = B // N_CORES      # 512
TT = B_SHARD // P           # 4 slots, b = t*128 + p
ELEM = 64                   # 64 f32 = 256B gathered per lookup
GW = NF * TT * ELEM         # 6656 floats per partition in G
IDXW = NF * 32              # 832 int16 idx cols (26 fields x 512/16)

# hdr1 layout (int32 cols): first 13 fields' idx source + small blocks
AH = 13 * 32                # 416 idx-source cols per half
B0, B1 = AH, AH + NF * TT   # sel ids, col f*4+t
P40, P41 = B1, B1 + 4       # [0,1,2,3]
WB0 = P41                   # w bits, b bits
HDRW = WB0 + 2              # 526
HDR2W = AH                  # hdr2: last 13 fields' idx source

F32 = mybir.dt.float32
I32 = mybir.dt.int32
I16 = mybir.dt.int16
AF = mybir.ActivationFunctionType
NQ = 4                      # SWDGE queues


def build_nc(n_queues: int = NQ) -> bass.Bass:
    nc = bacc.Bacc("TRN2", num_swdge_queues=n_queues)

    hdr_ext = nc.declare_dram_parameter("hdr", [P, HDRW], I32, isOutput=False)
    a1_ext = nc.declare_dram_parameter("a1", [16, AH], I32, isOutput=False)
    hdr2_ext = nc.declare_dram_parameter("hdr2", [P, HDR2W], I32, isOutput=False)
    tab_ext = nc.declare_dram_parameter("embed_tables", [NF * VOCAB, EMB], F32, isOutput=False)
    out_ext = nc.declare_dram_parameter("out", [P, TT], F32, isOutput=True)

    from contextlib import ExitStack

    with ExitStack() as ctx:
        hdr = ctx.enter_context(nc.sbuf_tensor([P, HDRW], I32))
        hdr2 = ctx.enter_context(nc.sbuf_tensor([P, HDR2W], I32))
        qidx = ctx.enter_context(nc.sbuf_tensor([P, IDXW], I16))
        qidx32 = ctx.enter_context(nc.sbuf_tensor([P, IDXW], I32))
        sel = ctx.enter_context(nc.sbuf_tensor([P, NF * TT], I32))
        m4 = ctx.enter_context(nc.sbuf_tensor([P, NF * TT * 4], F32))
        g = ctx.enter_context(nc.sbuf_tensor([P, GW], F32))
        gm = ctx.enter_context(nc.sbuf_tensor([P, GW], F32))
        sqs = ctx.enter_context(nc.sbuf_tensor([P, GW], F32))
        s = ctx.enter_context(nc.sbuf_tensor([P, TT * EMB], F32))
        sh = ctx.enter_context(nc.sbuf_tensor([P, TT * 2 * EMB], F32))
        qvh = ctx.enter_context(nc.sbuf_tensor([P, TT * 2], F32))
        s2 = ctx.enter_context(nc.sbuf_tensor([P, TT * EMB], F32))
        ss = ctx.enter_context(nc.sbuf_tensor([P, TT], F32))
        qv = ctx.enter_context(nc.sbuf_tensor([P, TT], F32))
        x = ctx.enter_context(nc.sbuf_tensor([P, TT], F32))
        y = ctx.enter_context(nc.sbuf_tensor([P, TT], F32))
        wh = ctx.enter_context(nc.sbuf_tensor([P, 1], F32))
        d_sem = ctx.enter_context(nc.semaphore("d_sem"))
        vq_sem = ctx.enter_context(nc.semaphore("vq_sem"))
        vq2_sem = ctx.enter_context(nc.semaphore("vq2_sem"))
        d2_sem = ctx.enter_context(nc.semaphore("d2_sem"))
        da_sem = ctx.enter_context(nc.semaphore("da_sem"))
        v_sem = ctx.enter_context(nc.semaphore("v_sem"))
        aq_sem = ctx.enter_context(nc.semaphore("aq_sem"))
        ay_sem = ctx.enter_context(nc.semaphore("ay_sem"))
        # per (queue, half) gather-completion sems, waited at final value only
        gqh = [
            [ctx.enter_context(nc.semaphore(f"gq{q}h{h}")) for h in range(2)]
            for q in range(NQ)
        ]
        # per (t, half) GM-ready sems
        ghsem = [
            [ctx.enter_context(nc.semaphore(f"gh{t}{h}")) for h in range(2)]
            for t in range(TT)
        ]
        block = ctx.enter_context(nc.Block())
        H0 = 13  # fields in group 0
        # work items (field, t0, nt, queue): fields 24/25 split into halves
        # so every queue carries 6.5 gather-slots
        WORK = [(f, 0, TT, f % NQ) for f in range(24)]
        WORK += [(24, 0, 2, 2), (24, 2, 2, 3), (25, 0, 2, 0), (25, 2, 2, 1)]
        gqh_count = [[0] * 2 for _ in range(NQ)]
        for f, t0, nt, qn in WORK:
            gqh_count[qn][0 if f < H0 else 1] += 1
        w_ap = hdr[:, WB0 : WB0 + 1].bitcast(F32)
        b_ap = hdr[:, WB0 + 1 : WB0 + 2].bitcast(F32)

        @block.sync
        def _(sync):
            sync.dma_start(
                out=hdr[:, 0:AH],
                in_=a1_ext[:].rearrange("(o b) c -> o b c", o=1).to_broadcast(
                    [8, 16, AH]
                ),
            ).then_inc(da_sem, 16)
            sync.dma_start(out=hdr[:, AH:], in_=hdr_ext[:, AH:]).then_inc(d_sem, 16)
            sync.dma_start(out=hdr2[:], in_=hdr2_ext[:]).then_inc(d2_sem, 16)
            sync.wait_ge(ay_sem, 1)
            sync.dma_start(out=out_ext[:], in_=y[:]).then_inc(d_sem, 16)
            sync.wait_ge(d_sem, 32)

        @block.vector
        def _(vector):
            vector.wait_ge(da_sem, 16)
            vector.tensor_scalar(
                out=qidx32[:, :AH],
                in0=hdr[:, 0:AH],
                scalar1=2,
                scalar2=None,
                op0=mybir.AluOpType.logical_shift_right,
            ).then_inc(v_sem, 1)  # v=1
            vector.wait_ge(v_sem, 1)
            vector.tensor_copy(qidx[:, :AH], qidx32[:, :AH]).then_inc(vq_sem, 1)
            vector.wait_ge(d_sem, 16)
            vector.tensor_scalar(
                out=sel[:],
                in0=hdr[:, B0:B1],
                scalar1=3,
                scalar2=None,
                op0=mybir.AluOpType.bitwise_and,
            ).then_inc(v_sem, 1)  # v=2
            vector.wait_ge(v_sem, 2)
            vector.tensor_tensor(
                out=m4[:],
                in0=sel[:].rearrange("p (c o) -> p c o", o=1).to_broadcast(
                    [P, NF * TT, 4]
                ),
                in1=hdr[:, P40:P41].rearrange("p (o r) -> p o r", o=1).to_broadcast(
                    [P, NF * TT, 4]
                ),
                op=mybir.AluOpType.is_equal,
            ).then_inc(v_sem, 1)  # v=3
            vector.wait_ge(d2_sem, 16)
            vector.tensor_scalar(
                out=qidx32[:, AH:],
                in0=hdr2[:],
                scalar1=2,
                scalar2=None,
                op0=mybir.AluOpType.logical_shift_right,
            ).then_inc(v_sem, 1)  # v=4
            vector.wait_ge(v_sem, 4)
            vector.tensor_copy(qidx[:, AH:], qidx32[:, AH:]).then_inc(vq2_sem, 1)
            # views: G (f t r k), GM' (t k f r), M4 (f t r)
            g5 = g[:].rearrange("p (f t r k) -> p t f r k", f=NF, t=TT, r=4, k=EMB)
            gm5 = gm[:].rearrange("p (t k f r) -> p t f r k", t=TT, k=EMB, f=NF, r=4)
            m45 = m4[:].rearrange("p (f t r) -> p t f r", f=NF, t=TT, r=4)
            gm_tkc = gm[:].rearrange(
                "p (t k c) -> p t k c", t=TT, k=EMB, c=NF * 4
            )
            vcount = 4
            for h in range(2):
                f0, f1 = (0, H0) if h == 0 else (H0, NF)
                nf_h = f1 - f0
                for q in range(NQ):
                    vector.wait_ge(gqh[q][h], 16 * gqh_count[q][h])
                for t in range(TT):
                    vector.tensor_tensor(
                        out=gm5[:, t : t + 1, f0:f1],
                        in0=g5[:, t : t + 1, f0:f1],
                        in1=m45[:, t : t + 1, f0:f1].to_broadcast(
                            [P, 1, nf_h, 4, EMB]
                        ),
                        op=mybir.AluOpType.mult,
                    ).then_inc(ghsem[t][h], 1)
                for t in range(TT):
                    vector.wait_ge(ghsem[t][h], 1)
                    vector.reduce_sum(
                        sh[:, (t * 2 + h) * EMB : (t * 2 + h + 1) * EMB].rearrange(
                            "p (o k) -> p o k", o=1
                        ),
                        gm_tkc[:, t : t + 1, :, (0 if h == 0 else H0) * 4 : (H0 if h == 0 else NF) * 4],
                        axis=mybir.AxisListType.X,
                    ).then_inc(v_sem, 1)
                    vcount += 1  # v=5..12
            vector.wait_ge(v_sem, 12)
            sh_v = sh[:].rearrange("p (t h k) -> p t h k", t=TT, h=2, k=EMB)
            vector.tensor_tensor(
                out=s[:].rearrange("p (t k) -> p t k", t=TT).rearrange(
                    "p t (o k) -> p t o k", o=1
                ),
                in0=sh_v[:, :, 0:1],
                in1=sh_v[:, :, 1:2],
                op=mybir.AluOpType.add,
            ).then_inc(v_sem, 1)  # v=13
            vector.wait_ge(v_sem, 13)
            vector.tensor_mul(s2[:], s[:], s[:]).then_inc(v_sem, 1)  # v=14
            vector.wait_ge(v_sem, 14)
            vector.reduce_sum(
                ss[:],
                s2[:].rearrange("p (t k) -> p t k", t=TT),
                axis=mybir.AxisListType.X,
            ).then_inc(v_sem, 1)  # v=15
            vector.tensor_scalar_mul(wh[:], w_ap, 0.5).then_inc(v_sem, 1)  # v=16
            vector.wait_ge(aq_sem, TT * 2)
            qvh_v = qvh[:].rearrange("p (t h) -> p t h", t=TT, h=2)
            vector.tensor_tensor(
                out=qv[:].rearrange("p (t o) -> p t o", o=1),
                in0=qvh_v[:, :, 0:1],
                in1=qvh_v[:, :, 1:2],
                op=mybir.AluOpType.add,
            ).then_inc(v_sem, 1)  # v=17
            vector.wait_ge(v_sem, 17)
            vector.tensor_tensor(
                x[:], ss[:], qv[:], op=mybir.AluOpType.subtract
            ).then_inc(v_sem, 1)  # v=18 (final)

        @block.gpsimd
        def _(gpsimd):
            gpsimd.load_library(library_config.mlp)
            gpsimd.wait_ge(vq_sem, 1)
            for f, t0, nt, qn in WORK:
                if f == 13 and t0 == 0:
                    gpsimd.wait_ge(vq2_sem, 1)
                slab = tab_ext[f * VOCAB : (f + 1) * VOCAB, :].rearrange(
                    "(a b) k -> a (b k)", b=4
                )  # [25000, 64] 256B rows
                gpsimd.dma_gather(
                    out_ap=g[:, f * TT * ELEM : (f + 1) * TT * ELEM].rearrange(
                        "p (t e) -> p t e", e=ELEM
                    )[:, t0 : t0 + nt, :],
                    in_ap=slab,
                    idxs_ap=qidx[:, f * 32 + t0 * 8 : f * 32 + (t0 + nt) * 8],
                    num_idxs=nt * P,
                    num_idxs_reg=nt * P,
                    elem_size=ELEM,
                    queue_num=qn,
                ).then_inc(gqh[qn][0 if f < H0 else 1], 16)

        @block.scalar
        def _(scalar):
            scalar.wait_ge(d_sem, 16)
            gm_a = gm[:].rearrange(
                "p (t k c) -> p t k c", t=TT, k=EMB, c=NF * 4
            )
            sqs_a = sqs[:].rearrange(
                "p (t k c) -> p t k c", t=TT, k=EMB, c=NF * 4
            )
            for h in range(2):
                c0, c1 = (0, H0 * 4) if h == 0 else (H0 * 4, NF * 4)
                for t in range(TT):
                    scalar.wait_ge(ghsem[t][h], 1)
                    scalar.activation(
                        sqs_a[:, t : t + 1, :, c0:c1],
                        gm_a[:, t : t + 1, :, c0:c1],
                        AF.Square,
                        accum_out=qvh[:, t * 2 + h : t * 2 + h + 1],
                    ).then_inc(aq_sem, 1)
            scalar.wait_ge(v_sem, 18)
            scalar.activation(
                y[:], x[:], AF.Sigmoid, bias=b_ap, scale=wh[:]
            ).then_inc(ay_sem, 1)

    nc.compile()
    return nc


_NC_CACHE = None


def _get_nc() -> bass.Bass:
    global _NC_CACHE
    if _NC_CACHE is None:
        _NC_CACHE = build_nc()
    return _NC_CACHE


def make_hdr(ids_shard: np.ndarray, w: np.float32, bb: np.float32):
    """-> (hdr1 [128,526], a1 [16,416], hdr2 [128,416]) int32."""
    a = np.zeros((16, NF * 32), dtype=np.int32)
    j = np.arange(B_SHARD)
    for f in range(NF):
        blk = np.zeros((16, 32), dtype=np.int32)
        blk[j % 16, j // 16] = ids_shard[:, f]
        a[:, f * 32 : (f + 1) * 32] = blk
    hdr = np.zeros((P, HDRW), dtype=np.int32)
    sel = ids_shard.reshape(TT, P, NF).transpose(1, 2, 0)  # [p, f, t]
    hdr[:, B0:B1] = sel.reshape(P, NF * TT)
    hdr[:, P40:P41] = np.arange(4, dtype=np.int32)[None, :]
    hdr[:, WB0 : WB0 + 2] = np.array([[w, bb]], dtype=np.float32).view(np.int32)
    return hdr, np.ascontiguousarray(a[:, :AH]), np.ascontiguousarray(np.tile(a[:, AH:], (8, 1)))


def make_in_maps(inputs: dict) -> list[dict]:
    ids = np.ascontiguousarray(np.asarray(inputs["sparse_ids"], dtype=np.int32))
    tab = np.ascontiguousarray(
        np.asarray(inputs["embed_tables"], dtype=np.float32)
    ).reshape(NF * VOCAB, EMB)
    w = np.float32(np.asarray(inputs["out_kernel"]).reshape(()))
    bb = np.float32(np.asarray(inputs["out_bias"]).reshape(()))
    maps = []
    for c in range(N_CORES):
        h1, a1, h2 = make_hdr(ids[c * B_SHARD : (c + 1) * B_SHARD], w, bb)
        maps.append({"hdr": h1, "a1": a1, "hdr2": h2, "embed_tables": tab})
    return maps


def run(inputs: dict, **spmd_kwargs):
    nc = _get_nc()
    in_maps = make_in_maps(inputs)
    res = run_bass_kernel_spmd(nc, in_maps, core_ids=list(range(N_CORES)), **spmd_kwargs)
    outs = []
    for i in range(N_CORES):
        yv = np.asarray(res.results[i]["out"], dtype=np.float32).reshape(P, TT)
        outs.append(yv.T.reshape(B_SHARD, 1))  # b = t*128 + p
    return np.concatenate(outs, axis=0), res


def kernel(**inputs) -> np.ndarray:
    out, _ = run(inputs)
    return out


# revision 27
# speedup vs baseline: 1.1706x; 1.1706x over previous
"""AFM forward on 8 TRN2 NeuronCores: dma_gather (256B quarter-row blocks) + on-chip select.

Math (softmax over the reference's size-1 axis is identically 1, so the
attention branch is dead): out[b] = sigmoid(0.5*w*(||S_b||^2 - Q_b) + bias) with
S_b = sum_f e_{b,f}, Q_b = sum_f ||e_{b,f}||^2.

Gather: per field f one InstDMAGatherAnt with int16 quarter-row indices
qid = id>>2 into table viewed as [25000, 64] 256B rows; each lookup lands
the 4-row block containing its embedding row.  Select on-chip with a 0/1
mask M4[(f,t),r] = (id & 3 == r).

Layouts: batch b = t*128 + p (t in [0,4)).  G/GM [128, f*256 + t*64 + r*16+k].
Host transposes the [128,4] output back to [512,1].
"""

import numpy as np

import concourse.bacc as bacc
import concourse.bass as bass
import concourse.mybir as mybir
from concourse import library_config
from concourse.bass_utils import run_bass_kernel_spmd

N_CORES = 8
B = 4096
NF = 26
EMB = 16
VOCAB = 100000
P = 128
B_SHARD = B // N_CORES      # 512
TT = B_SHARD // P           # 4 slots, b = t*128 + p
ELEM = 64                   # 64 f32 = 256B gathered per lookup
GW = NF * TT * ELEM         # 6656 floats per partition in G
IDXW = NF * 32              # 832 int16 idx cols (26 fields x 512/16)

# hdr1 layout (int32 cols): first 13 fields' idx source + small blocks
AH = 13 * 32                # 416 idx-source cols per half
B0, B1 = AH, AH + NF * TT   # sel ids, col f*4+t
P40, P41 = B1, B1 + 4       # [0,1,2,3]
WB0 = P41                   # w bits, b bits
HDRW = WB0 + 2              # 526
HDR2W = AH                  # hdr2: last 13 fields' idx source

F32 = mybir.dt.float32
I32 = mybir.dt.int32
I16 = mybir.dt.int16
AF = mybir.ActivationFunctionType
NQ = 4                      # SWDGE queues


def build_nc(n_queues: int = NQ) -> bass.Bass:
    nc = bacc.Bacc("TRN2", num_swdge_queues=n_queues, dynamic_dma_scratch_size=131072)

    hdr_ext = nc.declare_dram_parameter("hdr", [P, HDRW], I32, isOutput=False)
    hdr2_ext = nc.declare_dram_parameter("hdr2", [P, HDR2W], I32, isOutput=False)
    tab_ext = nc.declare_dram_parameter("embed_tables", [NF * VOCAB, EMB], F32, isOutput=False)
    out_ext = nc.declare_dram_parameter("out", [P, TT], F32, isOutput=True)

    from contextlib import ExitStack

    with ExitStack() as ctx:
        hdr = ctx.enter_context(nc.sbuf_tensor([P, HDRW], I32))
        hdr2 = ctx.enter_context(nc.sbuf_tensor([P, HDR2W], I32))
        qidx = ctx.enter_context(nc.sbuf_tensor([P, IDXW], I16))
        qidx32 = ctx.enter_context(nc.sbuf_tensor([P, IDXW], I32))
        sel = ctx.enter_context(nc.sbuf_tensor([P, NF * TT], I32))
        m4 = ctx.enter_context(nc.sbuf_tensor([P, NF * TT * 4], F32))
        g = ctx.enter_context(nc.sbuf_tensor([P, GW], F32))
        gm = ctx.enter_context(nc.sbuf_tensor([P, GW], F32))
        sqs = ctx.enter_context(nc.sbuf_tensor([P, GW], F32))
        s = ctx.enter_context(nc.sbuf_tensor([P, TT * EMB], F32))
        sh = ctx.enter_context(nc.sbuf_tensor([P, TT * 2 * EMB], F32))
        qvh = ctx.enter_context(nc.sbuf_tensor([P, TT * 2], F32))
        s2 = ctx.enter_context(nc.sbuf_tensor([P, TT * EMB], F32))
        ss = ctx.enter_context(nc.sbuf_tensor([P, TT], F32))
        qv = ctx.enter_context(nc.sbuf_tensor([P, TT], F32))
        x = ctx.enter_context(nc.sbuf_tensor([P, TT], F32))
        y = ctx.enter_context(nc.sbuf_tensor([P, TT], F32))
        wh = ctx.enter_context(nc.sbuf_tensor([P, 1], F32))
        d_sem = ctx.enter_context(nc.semaphore("d_sem"))
        vq_sem = ctx.enter_context(nc.semaphore("vq_sem"))
        vq2_sem = ctx.enter_context(nc.semaphore("vq2_sem"))
        d2_sem = ctx.enter_context(nc.semaphore("d2_sem"))
        v_sem = ctx.enter_context(nc.semaphore("v_sem"))
        aq_sem = ctx.enter_context(nc.semaphore("aq_sem"))
        ay_sem = ctx.enter_context(nc.semaphore("ay_sem"))
        # per (queue, half) gather-completion sems, waited at final value only
        gqh = [
            [ctx.enter_context(nc.semaphore(f"gq{q}h{h}")) for h in range(2)]
            for q in range(NQ)
        ]
        # per (t, half) GM-ready sems
        ghsem = [
            [ctx.enter_context(nc.semaphore(f"gh{t}{h}")) for h in range(2)]
            for t in range(TT)
        ]
        block = ctx.enter_context(nc.Block())
        H0 = 13  # fields in group 0
        # work items (field, t0, nt, queue): fields 24/25 split into halves
        # so every queue carries 6.5 gather-slots
        WORK = [(f, 0, TT, f % NQ) for f in range(24)]
        WORK += [(24, 0, 2, 2), (24, 2, 2, 3), (25, 0, 2, 0), (25, 2, 2, 1)]
        gqh_count = [[0] * 2 for _ in range(NQ)]
        for f, t0, nt, qn in WORK:
            gqh_count[qn][0 if f < H0 else 1] += 1
        w_ap = hdr[:, WB0 : WB0 + 1].bitcast(F32)
        b_ap = hdr[:, WB0 + 1 : WB0 + 2].bitcast(F32)

        @block.sync
        def _(sync):
            sync.dma_start(out=hdr[:], in_=hdr_ext[:]).then_inc(d_sem, 16)
            sync.dma_start(out=hdr2[:], in_=hdr2_ext[:]).then_inc(d2_sem, 16)
            sync.wait_ge(ay_sem, 1)
            sync.dma_start(out=out_ext[:], in_=y[:]).then_inc(d_sem, 16)
            sync.wait_ge(d_sem, 32)

        @block.vector
        def _(vector):
            vector.wait_ge(d_sem, 16)
            vector.tensor_scalar(
                out=qidx32[:, :AH],
                in0=hdr[:, 0:AH],
                scalar1=2,
                scalar2=None,
                op0=mybir.AluOpType.logical_shift_right,
            ).then_inc(v_sem, 1)  # v=1
            vector.wait_ge(v_sem, 1)
            vector.tensor_copy(qidx[:, :AH], qidx32[:, :AH]).then_inc(vq_sem, 1)
            vector.tensor_scalar(
                out=sel[:],
                in0=hdr[:, B0:B1],
                scalar1=3,
                scalar2=None,
                op0=mybir.AluOpType.bitwise_and,
            ).then_inc(v_sem, 1)  # v=2
            vector.wait_ge(v_sem, 2)
            vector.tensor_tensor(
                out=m4[:],
                in0=sel[:].rearrange("p (c o) -> p c o", o=1).to_broadcast(
                    [P, NF * TT, 4]
                ),
                in1=hdr[:, P40:P41].rearrange("p (o r) -> p o r", o=1).to_broadcast(
                    [P, NF * TT, 4]
                ),
                op=mybir.AluOpType.is_equal,
            ).then_inc(v_sem, 1)  # v=3
            vector.wait_ge(d2_sem, 16)
            vector.tensor_scalar(
                out=qidx32[:, AH:],
                in0=hdr2[:],
                scalar1=2,
                scalar2=None,
                op0=mybir.AluOpType.logical_shift_right,
            ).then_inc(v_sem, 1)  # v=4
            vector.wait_ge(v_sem, 4)
            vector.tensor_copy(qidx[:, AH:], qidx32[:, AH:]).then_inc(vq2_sem, 1)
            # views: G (f t r k), GM' (t k f r), M4 (f t r)
            g5 = g[:].rearrange("p (f t r k) -> p t f r k", f=NF, t=TT, r=4, k=EMB)
            gm5 = gm[:].rearrange("p (t k f r) -> p t f r k", t=TT, k=EMB, f=NF, r=4)
            m45 = m4[:].rearrange("p (f t r) -> p t f r", f=NF, t=TT, r=4)
            gm_tkc = gm[:].rearrange(
                "p (t k c) -> p t k c", t=TT, k=EMB, c=NF * 4
            )
            vcount = 4
            for h in range(2):
                f0, f1 = (0, H0) if h == 0 else (H0, NF)
                nf_h = f1 - f0
                for q in range(NQ):
                    vector.wait_ge(gqh[q][h], 16 * gqh_count[q][h])
                for t in range(TT):
                    vector.tensor_tensor(
                        out=gm5[:, t : t + 1, f0:f1],
                        in0=g5[:, t : t + 1, f0:f1],
                        in1=m45[:, t : t + 1, f0:f1].to_broadcast(
                            [P, 1, nf_h, 4, EMB]
                        ),
                        op=mybir.AluOpType.mult,
                    ).then_inc(ghsem[t][h], 1)
                for t in range(TT):
                    vector.wait_ge(ghsem[t][h], 1)
                    vector.reduce_sum(
                        sh[:, (t * 2 + h) * EMB : (t * 2 + h + 1) * EMB].rearrange(
                            "p (o k) -> p o k", o=1
                        ),
                        gm_tkc[:, t : t + 1, :, (0 if h == 0 else H0) * 4 : (H0 if h == 0 else NF) * 4],
                        axis=mybir.AxisListType.X,
                    ).then_inc(v_sem, 1)
                    vcount += 1  # v=5..12
            vector.wait_ge(v_sem, 12)
            sh_v = sh[:].rearrange("p (t h k) -> p t h k", t=TT, h=2, k=EMB)
            vector.tensor_tensor(
                out=s[:].rearrange("p (t k) -> p t k", t=TT).rearrange(
                    "p t (o k) -> p t o k", o=1
                ),
                in0=sh_v[:, :, 0:1],
                in1=sh_v[:, :, 1:2],
                op=mybir.AluOpType.add,
            ).then_inc(v_sem, 1)  # v=13
            vector.wait_ge(v_sem, 13)
            vector.tensor_mul(s2[:], s[:], s[:]).then_inc(v_sem, 1)  # v=14
            vector.wait_ge(v_sem, 14)
            vector.reduce_sum(
                ss[:],
                s2[:].rearrange("p (t k) -> p t k", t=TT),
                axis=mybir.AxisListType.X,
            ).then_inc(v_sem, 1)  # v=15
            vector.tensor_scalar_mul(wh[:], w_ap, 0.5).then_inc(v_sem, 1)  # v=16
            vector.wait_ge(aq_sem, TT * 2)
            qvh_v = qvh[:].rearrange("p (t h) -> p t h", t=TT, h=2)
            vector.tensor_tensor(
                out=qv[:].rearrange("p (t o) -> p t o", o=1),
                in0=qvh_v[:, :, 0:1],
                in1=qvh_v[:, :, 1:2],
                op=mybir.AluOpType.add,
            ).then_inc(v_sem, 1)  # v=17
            vector.wait_ge(v_sem, 17)
            vector.tensor_tensor(
                x[:], ss[:], qv[:], op=mybir.AluOpType.subtract
            ).then_inc(v_sem, 1)  # v=18 (final)

        @block.gpsimd
        def _(gpsimd):
            gpsimd.load_library(library_config.mlp)
            gpsimd.wait_ge(vq_sem, 1)
            for f, t0, nt, qn in WORK:
                if f == 13 and t0 == 0:
                    gpsimd.wait_ge(vq2_sem, 1)
                slab = tab_ext[f * VOCAB : (f + 1) * VOCAB, :].rearrange(
                    "(a b) k -> a (b k)", b=4
                )  # [25000, 64] 256B rows
                gpsimd.dma_gather(
                    out_ap=g[:, f * TT * ELEM : (f + 1) * TT * ELEM].rearrange(
                        "p (t e) -> p t e", e=ELEM
                    )[:, t0 : t0 + nt, :],
                    in_ap=slab,
                    idxs_ap=qidx[:, f * 32 + t0 * 8 : f * 32 + (t0 + nt) * 8],
                    num_idxs=nt * P,
                    num_idxs_reg=nt * P,
                    elem_size=ELEM,
                    queue_num=qn,
                    single_packet=False,
                ).then_inc(gqh[qn][0 if f < H0 else 1], 16)

        @block.scalar
        def _(scalar):
            scalar.wait_ge(d_sem, 16)
            gm_a = gm[:].rearrange(
                "p (t k c) -> p t k c", t=TT, k=EMB, c=NF * 4
            )
            sqs_a = sqs[:].rearrange(
                "p (t k c) -> p t k c", t=TT, k=EMB, c=NF * 4
            )
            for h in range(2):
                c0, c1 = (0, H0 * 4) if h == 0 else (H0 * 4, NF * 4)
                for t in range(TT):
                    scalar.wait_ge(ghsem[t][h], 1)
                    scalar.activation(
                        sqs_a[:, t : t + 1, :, c0:c1],
                        gm_a[:, t : t + 1, :, c0:c1],
                        AF.Square,
                        accum_out=qvh[:, t * 2 + h : t * 2 + h + 1],
                    ).then_inc(aq_sem, 1)
            scalar.wait_ge(v_sem, 18)
            scalar.activation(
                y[:], x[:], AF.Sigmoid, bias=b_ap, scale=wh[:]
            ).then_inc(ay_sem, 1)

    nc.compile()
    return nc


_NC_CACHE = None


def _get_nc() -> bass.Bass:
    global _NC_CACHE
    if _NC_CACHE is None:
        _NC_CACHE = build_nc()
    return _NC_CACHE


def make_hdr(ids_shard: np.ndarray, w: np.float32, bb: np.float32):
    """ids_shard [512, 26] int32 -> (hdr1 [128, 526], hdr2 [128, 416]) int32."""
    a = np.zeros((P, NF * 32), dtype=np.int32)
    j = np.arange(B_SHARD)
    for f in range(NF):
        blk = np.zeros((16, 32), dtype=np.int32)
        blk[j % 16, j // 16] = ids_shard[:, f]
        a[:, f * 32 : (f + 1) * 32] = np.tile(blk, (8, 1))
    hdr = np.zeros((P, HDRW), dtype=np.int32)
    hdr[:, 0:AH] = a[:, :AH]
    sel = ids_shard.reshape(TT, P, NF).transpose(1, 2, 0)  # [p, f, t]
    hdr[:, B0:B1] = sel.reshape(P, NF * TT)
    hdr[:, P40:P41] = np.arange(4, dtype=np.int32)[None, :]
    hdr[:, WB0 : WB0 + 2] = np.array([[w, bb]], dtype=np.float32).view(np.int32)
    return hdr, np.ascontiguousarray(a[:, AH:])


def make_in_maps(inputs: dict) -> list[dict]:
    ids = np.ascontiguousarray(np.asarray(inputs["sparse_ids"], dtype=np.int32))
    tab = np.ascontiguousarray(
        np.asarray(inputs["embed_tables"], dtype=np.float32)
    ).reshape(NF * VOCAB, EMB)
    w = np.float32(np.asarray(inputs["out_kernel"]).reshape(()))
    bb = np.float32(np.asarray(inputs["out_bias"]).reshape(()))
    maps = []
    for c in range(N_CORES):
        h1, h2 = make_hdr(ids[c * B_SHARD : (c + 1) * B_SHARD], w, bb)
        maps.append({"hdr": h1, "hdr2": h2, "embed_tables": tab})
    return maps


def run(inputs: dict, **spmd_kwargs):
    nc = _get_nc()
    in_maps = make_in_maps(inputs)
    res = run_bass_kernel_spmd(nc, in_maps, core_ids=list(range(N_CORES)), **spmd_kwargs)
    outs = []
    for i in range(N_CORES):
        yv = np.asarray(res.results[i]["out"], dtype=np.float32).reshape(P, TT)
        outs.append(yv.T.reshape(B_SHARD, 1))  # b = t*128 + p
    return np.concatenate(outs, axis=0), res


def kernel(**inputs) -> np.ndarray:
    out, _ = run(inputs)
    return out
